# revision 1
# baseline (speedup 1.0000x reference)
"""CQT magnitude kernel for Trainium2 (8 NeuronCores, Bass/Tile).

Strategy (v2)
-------------
C[k, n] = sum_l xpad[n*HOP + l] * kernel[k, l], regrouped over 128-wide
l-chunks: with X128[p, j] = xpad[j*128 + p] and HOP = 512 = 4*128,

    C[k, n] = sum_c sum_p kernel[k, c*128 + p] * X128[p, c + 4n]

One PE matmul per (l-chunk, re/im): lhsT = kernelT chunk ([128 l, M bins]),
rhs = strided view of X128, accumulated in PSUM across chunks.

v2 refinements over the 95us baseline (all exact, no numerics change):
 * M-trimmed weight loads: constant-Q support shrinks with frequency, so
   chunk c only has M_c active bins (a prefix, since bins are sorted by
   support).  LDWEIGHTS cost scales with weight *columns*, so loading
   [128, M_c] instead of [128, 128] cuts total LDWEIGHTS time ~2.5x.
 * Frame-trimmed matmuls: frames near the signal edges read all-zero
   padding chunks; per-slot the rhs is restricted to the valid frame
   window, cutting streamed matmul columns ~25%.
 * Strided chunk assignment (core q gets chunks 8s+q) keeps the frame
   windows SPMD-uniform (program identical across cores; all per-core
   variation lives in the packed inputs).
 * 3 kt DMA groups + 1 xi DMA + 4 output DMAs (vs 19 DMAs): fewer
   cross-engine semaphores shrink the fixed init/teardown storms.
 * PSUM init via two untrimmed full-coverage start=True slots per bin
   half; everything else accumulates with start=False.

Numerics: operands bf16 (PE streams 1 col/cycle), f32 PSUM accumulation;
host sums the 8 per-core partials and takes sqrt(re^2 + im^2).
"""

import numpy as np

# ---- problem constants (hardcoded per contract) ----
SR = 44100
BPO = 36
KBINS = 252
FMIN = 32.70319566257483
QF = 1.0 / (2.0 ** (1.0 / BPO) - 1.0)
SR_B, SR_TR, SR_T = 2, 2, 65536
NTRACKS = SR_B * SR_TR            # 4
L = 69376                          # filterbank window length
HOP = 512
PCH = 128
NCH = L // PCH                     # 542 l-chunks
NF = 1 + SR_T // HOP               # 129 frames
NCORES = 8
M1C0 = 247                         # first m1 chunk
NS0 = 68                           # m0 slots per core (542/8 rounded up)
NS1 = 6                           # m1 slots per core (48/8)
INIT0 = 33                         # untrimmed m0 init slot (center chunks)
J_VALID_LO, J_VALID_HI = 271, 782  # nonzero xpad chunk range (inclusive)
XPAD_CH = 1056
NWARM = 12

# ---- derived slot tables (exact sparsity of the constant-Q bank) ----


def _build_tables():
    freqs = FMIN * 2.0 ** (np.arange(KBINS) / BPO)
    lens = QF * SR / freqs
    lo = np.floor((L // 2 - lens / 2) / PCH).astype(int)
    hi = np.ceil((L // 2 + lens / 2) / PCH).astype(int)
    m0c = np.zeros(NCH + 8, int)
    m1c = np.zeros(NCH + 8, int)
    for k in range(128):
        m0c[lo[k] : hi[k]] = np.maximum(m0c[lo[k] : hi[k]], k + 1)
    for k in range(128, KBINS):
        m1c[lo[k] : hi[k]] = np.maximum(m1c[lo[k] : hi[k]], k - 127)
    m0s = np.array([max(m0c[8 * s + q] for q in range(8)) for s in range(NS0)])
    m1s = np.array(
        [max(m1c[M1C0 + 8 * s + q] for q in range(8)) for s in range(NS1)]
    )
    m0s = (m0s + 1) // 2 * 2
    m1s = (m1s + 1) // 2 * 2
    m0s[INIT0] = 128   # init slots cover every bin row (PSUM has_written)
    m1s[0] = 128

    def nrng(cl, ch):
        return max(0, -(-(J_VALID_LO - ch) // 4)), min(
            NF - 1, (J_VALID_HI - cl) // 4
        )

    f0 = [nrng(8 * s, 8 * s + 7) for s in range(NS0)]
    f1 = [nrng(M1C0 + 8 * s, M1C0 + 8 * s + 7) for s in range(NS1)]
    f0[INIT0] = (0, NF - 1)   # init slots untrimmed (full column coverage)
    f1[0] = (0, NF - 1)

    # emission order: m0 init (re, im), m0-re, m0-im, m1 init, m1-re, m1-im
    entries = []  # (m, s, part, M, n0, n1, start, stop)
    entries.append((0, INIT0, 0, 128, 0, NF - 1, True, False))
    entries.append((0, INIT0, 1, 128, 0, NF - 1, True, False))
    for part in range(2):
        for s in range(NS0):
            if s == INIT0:
                continue
            last = s == NS0 - 1
            entries.append(
                (0, s, part, int(m0s[s]), f0[s][0], f0[s][1], False, last)
            )
    entries.append((1, 0, 0, 128, 0, NF - 1, True, False))
    entries.append((1, 0, 1, 128, 0, NF - 1, True, False))
    for part in range(2):
        for s in range(1, NS1):
            last = s == NS1 - 1
            entries.append(
                (1, s, part, int(m1s[s]), f1[s][0], f1[s][1], False, last)
            )

    # kt column offsets + 3 DMA group split (~equal bytes after group 0)
    offs = np.cumsum([0] + [e[3] for e in entries])
    total = int(offs[-1])
    g0_end = 26                       # inits + the cheap small-M head
    rest = total - int(offs[g0_end])
    t1 = int(offs[g0_end]) + rest // 2
    g1_end = int(np.searchsorted(offs, t1))
    gsplits = [0, g0_end, g1_end, len(entries)]
    umax = 0
    for m, s, part, M, n0, n1, _, _ in entries:
        u0 = 8 * s + 4 * n0 - (M1C0 if m == 0 else 0)
        umax = max(umax, u0 + 4 * (n1 - n0))
    return entries, offs, gsplits, umax + 1


_ENTRIES, _KTOFF, _GSPL, _XIU = _build_tables()
_GCOLS = [int(_KTOFF[_GSPL[g + 1]] - _KTOFF[_GSPL[g]]) for g in range(3)]

_PROG = None


def _build_program():
    import concourse.bass as bass
    import concourse.mybir as mybir
    from concourse import bacc
    from concourse.tile import TileContext

    f32 = mybir.dt.float32
    bf16 = mybir.dt.bfloat16

    nc = bacc.Bacc(None, name="cqt_spmd2")
    kt_d = [
        nc.dram_tensor(f"kt{g}", [128, _GCOLS[g]], bf16, kind="ExternalInput")
        for g in range(3)
    ]
    xi_d = nc.dram_tensor("xi", [128, _XIU, 4], bf16, kind="ExternalInput")
    out_d = nc.dram_tensor("out", [128, 8 * 2 * NF], f32, kind="ExternalOutput")

    with TileContext(nc) as tc:
        with (
            tc.tile_pool(name="xip", bufs=1) as xip,
            tc.tile_pool(name="ktp", bufs=3) as ktp,
            tc.tile_pool(name="wp", bufs=1) as wp,
            tc.tile_pool(name="accp", bufs=1, space="PSUM") as accp,
        ):
            kt_t = []
            for g in range(3):
                t = ktp.tile([128, _GCOLS[g]], bf16, tag=f"kt{g}", name=f"kt{g}")
                nc.gpsimd.dma_start(out=t, in_=kt_d[g][:, :])
                kt_t.append(t)
                if g == 0:
                    xi_t = xip.tile([128, _XIU, 4], bf16)
                    nc.gpsimd.dma_start(out=xi_t, in_=xi_d[:, :, :])

            accs = [
                accp.tile([128, 2 * NF], f32, tag=f"acc{b}", name=f"acc{b}")
                for b in range(8)
            ]

            # PE pre-warm while the first DMAs land (HAM clock-gate opens
            # after ~3.4us of sustained PE activity).  Garbage lands in
            # bank 7, re-initialized by the m1-im start=True matmul later.
            wtile = wp.tile([128, 128], bf16)
            nc.vector.memset(wtile, 0.0)
            for _ in range(NWARM):
                nc.tensor.matmul(
                    accs[7][:, :128], wtile, wtile, start=True, stop=True
                )

            st = wp.tile([128, 8 * 2 * NF], f32, tag="st", name="st")

            g = 0
            flushed = 0
            for ei, (m, s, part, M, n0, n1, first, last) in enumerate(_ENTRIES):
                while ei >= _GSPL[g + 1]:
                    g += 1
                off = int(_KTOFF[ei] - _KTOFF[_GSPL[g]])
                lhsT = kt_t[g][:, off : off + M]
                u0 = 8 * s + 4 * n0 - (M1C0 if m == 0 else 0)
                F = n1 - n0 + 1
                for tp in range(2):
                    rhs = bass.AP(
                        tensor=xi_t.tensor,
                        offset=xi_t.offset + u0 * 4 + tp * 2,
                        ap=[xi_t.ap[0], [16, F], [1, 2]],
                    )
                    b = m * 4 + part * 2 + tp
                    out = accs[b][:M, 2 * n0 : 2 * n1 + 2]
                    nc.tensor.matmul(out, lhsT, rhs, start=first, stop=last)
                if last:
                    # this (m, part) pass is done: flush its two banks and,
                    # per pair, DMA the staged half out while later passes
                    # still stream on the PE
                    for tp in range(2):
                        b = m * 4 + part * 2 + tp
                        nc.vector.tensor_copy(
                            st[:, b * 2 * NF : (b + 1) * 2 * NF], accs[b]
                        )
                    lo = flushed * 2 * NF
                    hi = (flushed + 2) * 2 * NF
                    nc.gpsimd.dma_start(out=out_d[:, lo:hi], in_=st[:, lo:hi])
                    flushed += 2
    nc.finalize()
    _dedupe_ldweights(nc)
    return nc


def _dedupe_ldweights(nc):
    """Drop back-to-back InstLdweights with identical weights APs.

    The legalizer emits one LDWEIGHTS per MATMUL; consecutive matmuls that
    share lhsT (the two track-pair matmuls of each slot-part) reload the
    same weights.  LDWEIGHTS is a fixed ~115ns serialized on the PE weight
    path, so each removed reload is straight wall-clock.  Only duplicates
    carrying no semaphore waits/updates are dropped.
    """
    for fn in nc.m.functions:
        for bb in fn.blocks:
            insts = list(bb.instructions)
            keep = []
            prev_key = None
            for inst in insts:
                if type(inst).__name__ == 'InstLdweights':
                    key = str(inst.ins[0])
                    si = inst.sync_info
                    clean = not si or (
                        len(si.on_wait) == 0 and len(si.on_update) == 0
                    )
                    if key == prev_key and clean:
                        continue
                    prev_key = key
                keep.append(inst)
            if len(keep) != len(insts):
                bb.instructions = keep


def _pack_inputs(x, kr, ki):
    import ml_dtypes

    bf16 = ml_dtypes.bfloat16
    xf = np.ascontiguousarray(
        np.asarray(x, dtype=np.float32).reshape(NTRACKS, SR_T)
    )
    kr = np.asarray(kr, dtype=np.float32)
    ki = np.asarray(ki, dtype=np.float32)

    krT0 = np.ascontiguousarray(kr[:128].T)   # [L, 128]
    kiT0 = np.ascontiguousarray(ki[:128].T)

    def padT(mat):
        buf = np.zeros((128, L), np.float32)
        buf[: mat.shape[0]] = mat
        return np.ascontiguousarray(buf.T)

    krT1 = padT(kr[128:])
    kiT1 = padT(ki[128:])
    halves = {(0, 0): krT0, (0, 1): kiT0, (1, 0): krT1, (1, 1): kiT1}

    xpad = np.zeros((NTRACKS, XPAD_CH * PCH), np.float32)
    xpad[:, L // 2 : L // 2 + SR_T] = xf
    XI_full = np.ascontiguousarray(
        xpad.reshape(NTRACKS, XPAD_CH, PCH).transpose(2, 1, 0)
    )  # [128, 1056, 4]

    in_maps = []
    for q in range(NCORES):
        kt = np.zeros((128, int(_KTOFF[-1])), np.float32)
        for ei, (m, s, part, M, n0, n1, _, _) in enumerate(_ENTRIES):
            c = 8 * s + q + (M1C0 if m == 1 else 0)
            if c >= NCH + (M1C0 if m == 1 else 0) and m == 0:
                continue  # chunks 542/543 on cores 6-7: zero weights
            off = int(_KTOFF[ei])
            kt[:, off : off + M] = halves[(m, part)][c * 128 : (c + 1) * 128, :M]
        ktb = kt.astype(bf16)
        im = {
            f"kt{g}": np.ascontiguousarray(
                ktb[:, int(_KTOFF[_GSPL[g]]) : int(_KTOFF[_GSPL[g + 1]])]
            )
            for g in range(3)
        }
        im["xi"] = np.ascontiguousarray(
            XI_full[:, M1C0 + q : M1C0 + q + _XIU].astype(bf16)
        )
        in_maps.append(im)
    return in_maps


def _combine(outs):
    re_acc = np.zeros((KBINS, NTRACKS, NF), np.float32)
    im_acc = np.zeros((KBINS, NTRACKS, NF), np.float32)
    for q in range(NCORES):
        o = outs[q].reshape(128, 8, NF, 2)
        for b in range(8):
            m, part, tp = b >> 2, (b >> 1) & 1, b & 1
            rows = slice(0, 128) if m == 0 else slice(128, KBINS)
            nrows = 128 if m == 0 else KBINS - 128
            tgt = re_acc if part == 0 else im_acc
            tgt[rows, tp * 2 : (tp + 1) * 2] += o[:nrows, b].transpose(0, 2, 1)
    y = np.sqrt(re_acc**2 + im_acc**2)  # [252, 4, 129]
    return np.ascontiguousarray(
        y.reshape(KBINS, SR_B, SR_TR, NF).transpose(1, 0, 3, 2)
    )


def kernel(x, kr, ki):
    global _PROG
    from concourse.bass_utils import run_bass_kernel_spmd

    if _PROG is None:
        _PROG = _build_program()
    in_maps = _pack_inputs(x, kr, ki)
    res = run_bass_kernel_spmd(_PROG, in_maps, core_ids=list(range(NCORES)))
    outs = [res.results[q]["out"] for q in range(NCORES)]
    return _combine(outs)

